# revision 14
# baseline (speedup 1.0000x reference)
"""Trainium2 Bass kernel for a DoReFa-quantized DenseNet basic block.

Computes, for x:[128,256,32,32] f32:
  bn   = x * inv + (beta - mean*inv)          (inference BatchNorm)
  aq   = round(15 * clip(bn, 0, 1)) / 15      (4-bit activation quant, RNE)
  wq   = 2*round(15*wn)/15 - 1                (4-bit weight quant, host-side)
  conv = conv2d(aq, wq, 3x3, pad 1)
  out  = concat([x, conv], axis=1)            -> [128, 268, 32, 32]

Strategy: data-parallel over batch across 8 NeuronCores (16 images each).
The quantized activations are exact small integers 0..15 and quantized
weights are exact odd integers -15..15, so the conv runs on the PE array in
bf16 with *exact* integer arithmetic (fp32 PSUM accumulation), scaled by
1/225 on the way out.  The 3x3 conv is 18 PSUM-accumulated matmuls per
512-pixel chunk: one [128C x 12G] weight tap against a W-padded activation
tile with shifted access patterns (9 taps x 2 C-halves).
"""

from contextlib import ExitStack

import numpy as np
import ml_dtypes

import jax
import concourse.bass as bass
import concourse.tile as tile
from concourse import bacc, mybir
from concourse.bass2jax import _bass_exec_p, install_neuronx_cc_hook, partition_id_tensor
from jax.experimental.shard_map import shard_map
from jax.sharding import Mesh, PartitionSpec

N_CORES = 8
B, C, H, W = 128, 256, 32, 32
G = 12            # growthRate (conv output channels)
B_LOC = B // N_CORES
HW = H * W
BN_EPS = 1e-5
MAGIC = 8388608.0  # 2**23: adding then subtracting rounds fp32 to nearest int (RNE)

_CACHE: dict = {}


def _build_nc(xin_bufs=3, tmp_bufs=2, ps_bufs=4, cout_bufs=2):
    f32 = mybir.dt.float32
    fp8 = mybir.dt.float8e4
    nc = bacc.Bacc("TRN2", target_bir_lowering=False, debug=False, num_devices=N_CORES)

    x = nc.dram_tensor("x", [B_LOC, C, HW], f32, kind="ExternalInput")
    bn_scale = nc.dram_tensor("bn_scale", [128, 2], f32, kind="ExternalInput")
    bn_bias = nc.dram_tensor("bn_bias", [128, 2], f32, kind="ExternalInput")
    # [p, kh, kw, c_half, oc_padded(16)] — oc padded 12->16 so the DoubleRow
    # pair stride is a multiple of 16 elements
    wq = nc.dram_tensor("wq", [128, 3, 3, 2, 16], fp8, kind="ExternalInput")
    out = nc.dram_tensor("out", [B_LOC, C + G, HW], f32, kind="ExternalOutput")

    with ExitStack() as ctx:
        tc = ctx.enter_context(tile.TileContext(nc))
        singles = ctx.enter_context(tc.tile_pool(name="singles", bufs=1))
        xin = ctx.enter_context(tc.tile_pool(name="xin", bufs=xin_bufs))
        tmp = ctx.enter_context(tc.tile_pool(name="tmp", bufs=tmp_bufs))
        pspool = ctx.enter_context(tc.tile_pool(name="ps", bufs=ps_bufs, space="PSUM"))
        cout = ctx.enter_context(tc.tile_pool(name="cout", bufs=cout_bufs))

        w_tile = singles.tile([128, 3, 3, 2, 16], fp8)
        nc.sync.dma_start(out=w_tile[:], in_=wq[:])
        bns = singles.tile([128, 2], f32)
        nc.sync.dma_start(out=bns[:], in_=bn_scale[:])
        bnb = singles.tile([128, 2], f32)
        nc.sync.dma_start(out=bnb[:], in_=bn_bias[:])

        for pr in range(B_LOC // 2):
            b0 = pr * 2
            # channel c = 2p + g: per-partition DRAM chunk is one contiguous
            # 8 KB run per image; pairs of images per DMA amortize fixed costs
            x_tile = xin.tile([128, 2, 2, HW], f32)  # [p, img, g, hw]
            nc.sync.dma_start(
                out=x_tile[:],
                in_=x[b0 : b0 + 2].rearrange("b (p g) m -> p b g m", p=128),
            )
            # bn = relu(x*inv + shift)  (per-channel scale/bias, lower clip)
            t_tile = tmp.tile([128, 2, 2, HW], f32, tag="t")
            for g in range(2):
                nc.scalar.activation(
                    out=t_tile[:, :, g],
                    in_=x_tile[:, :, g],
                    func=mybir.ActivationFunctionType.Relu,
                    bias=bnb[:, g : g + 1],
                    scale=bns[:, g : g + 1],
                )
            # u = 15*min(bn,1) + 2^23   (upper clip, scale, begin RNE round)
            u_tile = tmp.tile([128, 2, 2, HW], f32, tag="u")
            nc.vector.tensor_scalar(
                u_tile[:],
                t_tile[:],
                1.0,
                15.0,
                mybir.AluOpType.min,
                mybir.AluOpType.mult,
            )
            # a = (u + 2^23) - 2^23 -> integer 0..15, cast fp8 (exact)
            a_tile = tmp.tile([128, 2, 2, HW], fp8, tag="a")
            nc.vector.tensor_scalar(
                a_tile[:],
                u_tile[:],
                MAGIC,
                MAGIC,
                mybir.AluOpType.add,
                mybir.AluOpType.subtract,
            )
            # 3x3 conv via 9 DoubleRow (K=256) PSUM-accumulated matmuls per
            # 512-pixel chunk; H and W edge taps are clipped (zero padding)
            co = cout.tile([G, 2, HW], f32)
            for im in range(2):
                a_view = a_tile[:, im].rearrange("p g (h w) -> p g h w", w=W)
                for ch in range(2):
                    h0 = ch * 16
                    ps = pspool.tile([G, 512], f32)
                    ps_view = ps[:].rearrange("p (h w) -> p h w", w=W)
                    taps = [(dh, dw) for dh in (0, -1, 1) for dw in (-1, 0, 1)]
                    for i, (dh, dw) in enumerate(taps):
                        hlo = max(h0, -dh)
                        hhi = min(h0 + 16, H - dh)
                        wlo = max(0, -dw)
                        whi = min(W, W - dw)
                        rhs = a_view[:, :, hlo + dh : hhi + dh, wlo + dw : whi + dw]
                        nc.tensor.matmul(
                            ps_view[:, hlo - h0 : hhi - h0, wlo:whi],
                            w_tile[:, dh + 1, dw + 1, :, 0:G],
                            rhs,
                            start=(i == 0),
                            stop=(i == len(taps) - 1),
                            perf_mode=mybir.MatmulPerfMode.DoubleRow,
                            skip_group_check=True,
                        )
                    nc.scalar.activation(
                        out=co[:, im, ch * 512 : (ch + 1) * 512],
                        in_=ps[:],
                        func=mybir.ActivationFunctionType.Copy,
                        scale=1.0 / 225.0,
                    )
            nc.sync.dma_start(
                out=out[b0 : b0 + 2, 0:C].rearrange("b (p g) m -> p b g m", p=128),
                in_=x_tile[:],
            )
            nc.gpsimd.dma_start(
                out=out[b0 : b0 + 2, C : C + G].rearrange("b c m -> c b m"),
                in_=co[:],
            )
    nc.compile()
    return nc


def _get_runner():
    """Build (once) a jitted 8-core sharded executor for the bass kernel.

    Mirrors bass2jax.run_bass_via_pjrt's multi-core branch, but caches the
    jitted callable so repeated kernel() calls don't re-trace/re-compile.
    No donation: the kernel writes every output element.
    """
    if "runner" in _CACHE:
        return _CACHE["runner"]

    install_neuronx_cc_hook()
    nc = _build_nc()
    partition_name = nc.partition_id_tensor.name if nc.partition_id_tensor else None

    in_names: list[str] = []
    out_names: list[str] = []
    out_avals: list[jax.core.ShapedArray] = []
    zero_outs: list[np.ndarray] = []
    for alloc in nc.m.functions[0].allocations:
        if not isinstance(alloc, mybir.MemoryLocationSet):
            continue
        name = alloc.memorylocations[0].name
        if alloc.kind == "ExternalInput":
            if name != partition_name:
                in_names.append(name)
        elif alloc.kind == "ExternalOutput":
            shape = tuple(alloc.tensor_shape)
            dtype = mybir.dt.np(alloc.dtype)
            out_names.append(name)
            out_avals.append(jax.core.ShapedArray(shape, dtype))
            zero_outs.append(np.zeros(shape, dtype))
    n_params = len(in_names)
    all_in_names = in_names + out_names
    if partition_name is not None:
        all_in_names = all_in_names + [partition_name]

    def _body(*args):
        operands = list(args)
        if partition_name is not None:
            operands.append(partition_id_tensor())
        outs = _bass_exec_p.bind(
            *operands,
            out_avals=tuple(out_avals),
            in_names=tuple(all_in_names),
            out_names=tuple(out_names),
            lowering_input_output_aliases=(),
            sim_require_finite=True,
            sim_require_nnan=True,
            nc=nc,
        )
        return tuple(outs)

    devices = jax.devices()[:N_CORES]
    mesh = Mesh(np.asarray(devices), ("core",))
    n_outs = len(out_names)
    sharded = jax.jit(
        shard_map(
            _body,
            mesh=mesh,
            in_specs=(PartitionSpec("core"),) * (n_params + n_outs),
            out_specs=(PartitionSpec("core"),) * n_outs,
            check_rep=False,
        ),
        keep_unused=True,
    )
    runner = (sharded, in_names, out_names, zero_outs)
    _CACHE["runner"] = runner
    return runner


def _host_prep(x, gamma, beta, mean, var, weight):
    """Host-side prep: fold BN params, quantize the tiny conv weight."""
    inv = (gamma / np.sqrt(var + BN_EPS)).astype(np.float32)
    shift = (beta - mean * inv).astype(np.float32)
    bn_scale = inv.reshape(128, 2).copy()  # [p, g] with c = 2p + g
    bn_bias = shift.reshape(128, 2).copy()

    # DoReFa weight quant (forward value): wq = 2*round(15*wn)/15 - 1,
    # wn = tanh(w)/(2*max|tanh(w)|) + 0.5.  Stored as integer 15*wq.
    t = np.tanh(weight.astype(np.float32))
    wn = t / (2.0 * np.abs(t).max()) + np.float32(0.5)
    q15 = np.round(wn * np.float32(15.0))
    w_int = (2.0 * q15 - 15.0).astype(np.float32)  # [G, C, 3, 3], odd ints
    # lhsT layout [p, kh, kw, j, oc_pad16] with c = 2p + j; odd ints <=15 are
    # exact in e4m3
    wq_l = np.zeros((128, 3, 3, 2, 16), np.float32)
    wq_l[:, :, :, :, :G] = w_int.reshape(G, 128, 2, 3, 3).transpose(1, 3, 4, 2, 0)
    wq_l = wq_l.astype(ml_dtypes.float8_e4m3)
    return bn_scale, bn_bias, wq_l


def kernel(x, gamma, beta, mean, var, weight):
    x = np.asarray(x, dtype=np.float32)
    bn_scale, bn_bias, wq_l = _host_prep(
        x,
        np.asarray(gamma, np.float32),
        np.asarray(beta, np.float32),
        np.asarray(mean, np.float32),
        np.asarray(var, np.float32),
        np.asarray(weight, np.float32),
    )
    sharded, in_names, out_names, zero_outs = _get_runner()

    x3 = x.reshape(B, C, HW)  # batch-sharded: core c gets rows [16c, 16c+16)
    per_input = {
        "x": x3,
        "bn_scale": np.concatenate([bn_scale] * N_CORES, axis=0),
        "bn_bias": np.concatenate([bn_bias] * N_CORES, axis=0),
        "wq": np.concatenate([wq_l] * N_CORES, axis=0),
    }
    concat_in = [per_input[name] for name in in_names]
    concat_zeros = [
        np.zeros((N_CORES * z.shape[0], *z.shape[1:]), z.dtype) for z in zero_outs
    ]
    out_arrs = sharded(*concat_in, *concat_zeros)
    out = np.asarray(out_arrs[out_names.index("out")])  # [B, C+G, HW]
    return out.reshape(B, C + G, H, W)


# revision 16
# speedup vs baseline: 18.2710x; 18.2710x over previous
"""Trainium2 Bass kernel for a DoReFa-quantized DenseNet basic block.

Computes, for x:[128,256,32,32] f32:
  bn   = x * inv + (beta - mean*inv)          (inference BatchNorm)
  aq   = round(15 * clip(bn, 0, 1)) / 15      (4-bit activation quant, RNE)
  wq   = 2*round(15*wn)/15 - 1                (4-bit weight quant, host-side)
  conv = conv2d(aq, wq, 3x3, pad 1)
  out  = concat([x, conv], axis=1)            -> [128, 268, 32, 32]

Strategy: data-parallel over batch across 8 NeuronCores (16 images each).
The quantized activations are exact small integers 0..15 and quantized
weights are exact odd integers -15..15, so the conv runs on the PE array in
bf16 with *exact* integer arithmetic (fp32 PSUM accumulation), scaled by
1/225 on the way out.  The 3x3 conv is 18 PSUM-accumulated matmuls per
512-pixel chunk: one [128C x 12G] weight tap against a W-padded activation
tile with shifted access patterns (9 taps x 2 C-halves).
"""

from contextlib import ExitStack

import numpy as np
import ml_dtypes

import jax
import concourse.bass as bass
import concourse.tile as tile
from concourse import bacc, mybir
from concourse.bass2jax import _bass_exec_p, install_neuronx_cc_hook, partition_id_tensor
from jax.experimental.shard_map import shard_map
from jax.sharding import Mesh, PartitionSpec

N_CORES = 8
B, C, H, W = 128, 256, 32, 32
G = 12            # growthRate (conv output channels)
B_LOC = B // N_CORES
HW = H * W
BN_EPS = 1e-5
MAGIC = 8388608.0  # 2**23: adding then subtracting rounds fp32 to nearest int (RNE)

_CACHE: dict = {}


def _build_nc(xin_bufs=3, tmp_bufs=2, ps_bufs=4, cout_bufs=2, reps=1):
    f32 = mybir.dt.float32
    fp8 = mybir.dt.float8e4
    nc = bacc.Bacc("TRN2", target_bir_lowering=False, debug=False, num_devices=N_CORES)

    x = nc.dram_tensor("x", [B_LOC, C, HW], f32, kind="ExternalInput")
    bn_scale = nc.dram_tensor("bn_scale", [128, 2], f32, kind="ExternalInput")
    bn_bias = nc.dram_tensor("bn_bias", [128, 2], f32, kind="ExternalInput")
    # [p, kh, kw, c_half, oc_padded(16)] — oc padded 12->16 so the DoubleRow
    # pair stride is a multiple of 16 elements
    wq = nc.dram_tensor("wq", [128, 3, 3, 2, 16], fp8, kind="ExternalInput")
    out = nc.dram_tensor("out", [B_LOC, C + G, HW], f32, kind="ExternalOutput")

    with ExitStack() as ctx:
        tc = ctx.enter_context(tile.TileContext(nc))
        singles = ctx.enter_context(tc.tile_pool(name="singles", bufs=1))
        xin = ctx.enter_context(tc.tile_pool(name="xin", bufs=xin_bufs))
        tmp = ctx.enter_context(tc.tile_pool(name="tmp", bufs=tmp_bufs))
        pspool = ctx.enter_context(tc.tile_pool(name="ps", bufs=ps_bufs, space="PSUM"))
        cout = ctx.enter_context(tc.tile_pool(name="cout", bufs=cout_bufs))

        w_tile = singles.tile([128, 3, 3, 2, 16], fp8)
        nc.sync.dma_start(out=w_tile[:], in_=wq[:])
        bns = singles.tile([128, 2], f32)
        nc.sync.dma_start(out=bns[:], in_=bn_scale[:])
        bnb = singles.tile([128, 2], f32)
        nc.sync.dma_start(out=bnb[:], in_=bn_bias[:])

        for rep_pr in range(reps * (B_LOC // 2)):
            b0 = (rep_pr % (B_LOC // 2)) * 2
            # channel c = 2p + g: per-partition DRAM chunk is one contiguous
            # 8 KB run per image; pairs of images per DMA amortize fixed costs
            x_tile = xin.tile([128, 2, 2, HW], f32)  # [p, img, g, hw]
            nc.sync.dma_start(
                out=x_tile[:],
                in_=x[b0 : b0 + 2].rearrange("b (p g) m -> p b g m", p=128),
            )
            # bn = relu(x*inv + shift)  (per-channel scale/bias, lower clip)
            t_tile = tmp.tile([128, 2, 2, HW], f32, tag="t")
            for g in range(2):
                nc.scalar.activation(
                    out=t_tile[:, :, g],
                    in_=x_tile[:, :, g],
                    func=mybir.ActivationFunctionType.Relu,
                    bias=bnb[:, g : g + 1],
                    scale=bns[:, g : g + 1],
                )
            # u = 15*min(bn,1) + 2^23   (upper clip, scale, begin RNE round)
            u_tile = tmp.tile([128, 2, 2, HW], f32, tag="u")
            nc.vector.tensor_scalar(
                u_tile[:],
                t_tile[:],
                1.0,
                15.0,
                mybir.AluOpType.min,
                mybir.AluOpType.mult,
            )
            # a = (u + 2^23) - 2^23 -> integer 0..15, cast fp8 (exact)
            a_tile = tmp.tile([128, 2, 2, HW], fp8, tag="a")
            nc.vector.tensor_scalar(
                a_tile[:],
                u_tile[:],
                MAGIC,
                MAGIC,
                mybir.AluOpType.add,
                mybir.AluOpType.subtract,
            )
            # 3x3 conv via 9 DoubleRow (K=256) PSUM-accumulated matmuls per
            # 512-pixel chunk; H and W edge taps are clipped (zero padding)
            co = cout.tile([G, 2, HW], f32)
            for im in range(2):
                a_view = a_tile[:, im].rearrange("p g (h w) -> p g h w", w=W)
                for ch in range(2):
                    h0 = ch * 16
                    ps = pspool.tile([G, 512], f32)
                    ps_view = ps[:].rearrange("p (h w) -> p h w", w=W)
                    taps = [(dh, dw) for dh in (0, -1, 1) for dw in (-1, 0, 1)]
                    for i, (dh, dw) in enumerate(taps):
                        hlo = max(h0, -dh)
                        hhi = min(h0 + 16, H - dh)
                        wlo = max(0, -dw)
                        whi = min(W, W - dw)
                        rhs = a_view[:, :, hlo + dh : hhi + dh, wlo + dw : whi + dw]
                        nc.tensor.matmul(
                            ps_view[:, hlo - h0 : hhi - h0, wlo:whi],
                            w_tile[:, dh + 1, dw + 1, :, 0:G],
                            rhs,
                            start=(i == 0),
                            stop=(i == len(taps) - 1),
                            perf_mode=mybir.MatmulPerfMode.DoubleRow,
                            skip_group_check=True,
                        )
                    nc.scalar.activation(
                        out=co[:, im, ch * 512 : (ch + 1) * 512],
                        in_=ps[:],
                        func=mybir.ActivationFunctionType.Copy,
                        scale=1.0 / 225.0,
                    )
            nc.sync.dma_start(
                out=out[b0 : b0 + 2, 0:C].rearrange("b (p g) m -> p b g m", p=128),
                in_=x_tile[:],
            )
            nc.gpsimd.dma_start(
                out=out[b0 : b0 + 2, C : C + G].rearrange("b c m -> c b m"),
                in_=co[:],
            )
    nc.compile()
    return nc


def _get_runner():
    """Build (once) a jitted 8-core sharded executor for the bass kernel.

    Mirrors bass2jax.run_bass_via_pjrt's multi-core branch, but caches the
    jitted callable so repeated kernel() calls don't re-trace/re-compile.
    No donation: the kernel writes every output element.
    """
    if "runner" in _CACHE:
        return _CACHE["runner"]

    install_neuronx_cc_hook()
    nc = _build_nc()
    partition_name = nc.partition_id_tensor.name if nc.partition_id_tensor else None

    in_names: list[str] = []
    out_names: list[str] = []
    out_avals: list[jax.core.ShapedArray] = []
    zero_outs: list[np.ndarray] = []
    for alloc in nc.m.functions[0].allocations:
        if not isinstance(alloc, mybir.MemoryLocationSet):
            continue
        name = alloc.memorylocations[0].name
        if alloc.kind == "ExternalInput":
            if name != partition_name:
                in_names.append(name)
        elif alloc.kind == "ExternalOutput":
            shape = tuple(alloc.tensor_shape)
            dtype = mybir.dt.np(alloc.dtype)
            out_names.append(name)
            out_avals.append(jax.core.ShapedArray(shape, dtype))
            zero_outs.append(np.zeros(shape, dtype))
    n_params = len(in_names)
    all_in_names = in_names + out_names
    if partition_name is not None:
        all_in_names = all_in_names + [partition_name]

    def _body(*args):
        operands = list(args)
        if partition_name is not None:
            operands.append(partition_id_tensor())
        outs = _bass_exec_p.bind(
            *operands,
            out_avals=tuple(out_avals),
            in_names=tuple(all_in_names),
            out_names=tuple(out_names),
            lowering_input_output_aliases=(),
            sim_require_finite=True,
            sim_require_nnan=True,
            nc=nc,
        )
        return tuple(outs)

    devices = jax.devices()[:N_CORES]
    mesh = Mesh(np.asarray(devices), ("core",))
    n_outs = len(out_names)
    sharded = jax.jit(
        shard_map(
            _body,
            mesh=mesh,
            in_specs=(PartitionSpec("core"),) * (n_params + n_outs),
            out_specs=(PartitionSpec("core"),) * n_outs,
            check_rep=False,
        ),
        keep_unused=True,
    )
    runner = (sharded, in_names, out_names, zero_outs)
    _CACHE["runner"] = runner
    return runner


def _host_prep(x, gamma, beta, mean, var, weight):
    """Host-side prep: fold BN params, quantize the tiny conv weight."""
    inv = (gamma / np.sqrt(var + BN_EPS)).astype(np.float32)
    shift = (beta - mean * inv).astype(np.float32)
    bn_scale = inv.reshape(128, 2).copy()  # [p, g] with c = 2p + g
    bn_bias = shift.reshape(128, 2).copy()

    # DoReFa weight quant (forward value): wq = 2*round(15*wn)/15 - 1,
    # wn = tanh(w)/(2*max|tanh(w)|) + 0.5.  Stored as integer 15*wq.
    t = np.tanh(weight.astype(np.float32))
    wn = t / (2.0 * np.abs(t).max()) + np.float32(0.5)
    q15 = np.round(wn * np.float32(15.0))
    w_int = (2.0 * q15 - 15.0).astype(np.float32)  # [G, C, 3, 3], odd ints
    # lhsT layout [p, kh, kw, j, oc_pad16] with c = 2p + j; odd ints <=15 are
    # exact in e4m3
    wq_l = np.zeros((128, 3, 3, 2, 16), np.float32)
    wq_l[:, :, :, :, :G] = w_int.reshape(G, 128, 2, 3, 3).transpose(1, 3, 4, 2, 0)
    wq_l = wq_l.astype(ml_dtypes.float8_e4m3)
    return bn_scale, bn_bias, wq_l


def kernel(x, gamma, beta, mean, var, weight):
    x = np.asarray(x, dtype=np.float32)
    bn_scale, bn_bias, wq_l = _host_prep(
        x,
        np.asarray(gamma, np.float32),
        np.asarray(beta, np.float32),
        np.asarray(mean, np.float32),
        np.asarray(var, np.float32),
        np.asarray(weight, np.float32),
    )
    sharded, in_names, out_names, zero_outs = _get_runner()

    x3 = x.reshape(B, C, HW)  # batch-sharded: core c gets rows [16c, 16c+16)
    per_input = {
        "x": x3,
        "bn_scale": np.concatenate([bn_scale] * N_CORES, axis=0),
        "bn_bias": np.concatenate([bn_bias] * N_CORES, axis=0),
        "wq": np.concatenate([wq_l] * N_CORES, axis=0),
    }
    concat_in = [per_input[name] for name in in_names]
    concat_zeros = [
        np.zeros((N_CORES * z.shape[0], *z.shape[1:]), z.dtype) for z in zero_outs
    ]
    out_arrs = sharded(*concat_in, *concat_zeros)
    out = np.asarray(out_arrs[out_names.index("out")])  # [B, C+G, HW]
    return out.reshape(B, C + G, H, W)
